# revision 33
# baseline (speedup 1.0000x reference)
"""Trainium2 Bass kernel for nn_HVGuardModel (dense MoE routing).

Reference math (B=65536, D=1024, E=8, H=128, C1=64, NC=2):
    gw  = softmax(x @ Wg + bg)                      [B, E]
    h   = relu(einsum('bd,edh', x, We1) + be1)      [B, E, H]
    eo  = einsum('beh,eho', h, We2) + be2           [B, E, H]
    mix = einsum('be,beh', gw, eo)                  [B, H]
    out = relu(mix @ Wc1 + bc1) @ Wc2 + bc2         [B, NC]

Strategy: pure data-parallel over 8 cores (8192 rows each), feature-major
[feature, batch] layout, zero device transposes, ALL-BF16 matmuls.

Why all-bf16 (v2 rewrite of the fp32r kernel, measured 351.5us):
  * fp32r matmuls run with fp32_mode=HIGH, which disables the PE's Fast
    Weight Load (EnableFWL requires in_dtype != FP32); the NTFF trace
    showed LDWEIGHTS at ~187 ns/matmul and a steady matmul pace of 233 ns
    vs the 213 ns streaming floor (512 cols @ 2.4 GHz).  bf16 matmuls are
    the same 1 col/cycle but FWL loads weights 2 elems/cycle and the PE's
    64-deep reorder window hides them entirely.
  * The old kernel uploaded x as bf16 and UPCAST to fp32r on DVE (one
    tensor_scalar per chunk).  The trace showed the tile-start gate and
    m=0 matmul stalls (~1 us/tile) all waiting on $S[162] = that DVE
    upcast semaphore, with DVE backed up behind a 3.3 us [64,512]
    RECIPROCAL.  bf16 matmuls consume the DMA'd chunks directly.
  * fp8/DoubleRow is a dead end on this HW: DR is only ~1.44x over bf16
    (LDWEIGHTS +72%, MATMUL +13%), and accuracy needs a hi/lo split that
    multiplies matmul count by >=2.  (Earlier fp32r-session conclusion,
    confirmed by the tensor-engine doc.)

Algebraic folds (host side):
  * V = We2 @ Wc1 per expert ([E*H, 64]) folds expert-2 + gate-mix +
    cls-1 into one PSUM accumulation; eo and mix are never materialized.
  * Layer-1 features interleaved f = j*E + e; a replicated-gate weight
    block (Wg columns tiled mod 8) gives per-partition gate scales with
    no cross-partition broadcast.
  * Softmax division deferred PAST cls-1 via relu(pp/s + bc1) =
    relu(pp + s*bc1)/s  (s > 0): the s*bc1 term rides the same [8,128]
    matmul on expg that applies Cm = be2 @ Wc1 (stationary += 1x8 outer
    bc1), and that merged matmul ALSO replicates s itself into PSUM rows
    64:66 (stationary cols 64:66 = 1).  So one [8,128]-stationary matmul
    replaces the old srep + C pair, and the division shrinks from a
    [64,512] DVE reciprocal (3.3 us!) + [64,512] multiply to a [2,512]
    reciprocal_approx_fast (~0.7 us, 18-bit exact) + [2,512] multiply on
    the final classifier output:  out = (Wc2.T relu(pp'))*(1/s) + bc2.

Schedule (per 512-column batch tile, 82 matmuls):
  * x chunk DMAs ride the Activation HWDGE queue, weights/outputs SP's;
    xpool bufs=3 lets the x stream run tiles ahead.
  * The classifier head is software-pipelined one tile behind.  Emission
    order keeps every engine's FIFO unblocked: rv (DVE recip of prev s),
    rp (ACT relu of prev pp, runs during gate matmuls), gate matmuls,
    expg (ACT, right after gate so the m-loop never waits), po matmul,
    out mul/bias/DMA, then the m-loop with the merged expg-matmul after
    block 0 and V matmuls trailing their hs production by 3 blocks.
"""

import numpy as np

B = 65536
D = 1024
E = 8
H = 128
C1 = 64
NCLS = 2
NCORES = 8
BLOC = B // NCORES  # 8192
NTILE = 512
KD = D // 128  # 8 k-chunks over D
MH = (E * H) // 128  # 8 feature blocks

MM_DT = "bfloat16"

_BUILT = {}


def _build_nc(b_per_core: int, repeat: int = 1):
    """repeat > 1 wraps the batch loop in a hardware For_i that re-runs the
    identical work `repeat` times -- used only for wall-clock timing."""
    import concourse.bacc as bacc
    import concourse.tile as tile
    import concourse.mybir as mybir

    nbt = b_per_core // NTILE
    fp32 = mybir.dt.float32
    bf16 = mybir.dt.bfloat16
    AF = mybir.ActivationFunctionType
    OP = mybir.AluOpType

    nc = bacc.Bacc("TRN2", target_bir_lowering=False, debug=False)

    xT = nc.dram_tensor("xT", [D, b_per_core], bf16, kind="ExternalInput")
    w1 = nc.dram_tensor("W1T", [128, MH * KD * 128], bf16, kind="ExternalInput")
    wg = nc.dram_tensor("WGT", [128, KD * 128], bf16, kind="ExternalInput")
    # V blocks padded to 128 stationary columns (cols 64:128 = 0) so every
    # pre-group matmul is a uniform K=128/M=128 shape: M=64 matmuls showed
    # col_grp=h0 array reconfig (+~190 ns each) and no FWL in the trace.
    vb = nc.dram_tensor("Vb", [128, MH * 128], bf16, kind="ExternalInput")
    # K=128 stationary against the mod-8-replicated expg: rows k = row
    # k%8 scaled by 1/16.  cols 0:64 = Cm + 1x8 (x) bc1 (pre term),
    # cols 64:66 = 1 (softmax denominator replicate into pp rows 64:66 --
    # legal as one accumulation group because the padded V matmuls write
    # the full [0:128] partition range), cols 66:128 = 0.
    cm = nc.dram_tensor("CMB", [128, 128], bf16, kind="ExternalInput")
    # Wc2 padded to 128 stationary columns (cols 2:128 = 0) for FWL and a
    # uniform M=128 shape; po rows 2:128 are zeros.
    wc2 = nc.dram_tensor("WC2", [C1, 128], bf16, kind="ExternalInput")
    # per-partition bias columns (fp32): 0..7 = be1 block m, 8 = bg_rep,
    # 9 = bc2 (rows 0:2)
    bcol = nc.dram_tensor("BCOL", [128, 10], fp32, kind="ExternalInput")
    yT = nc.dram_tensor("yT", [NCLS, b_per_core], fp32, kind="ExternalOutput")

    with tile.TileContext(nc) as tc:
        with (
            tc.tile_pool(name="wpool", bufs=1) as wpool,
            tc.tile_pool(name="xpool", bufs=2) as xpool,
            tc.tile_pool(name="spool", bufs=2) as spool,
            tc.tile_pool(name="hpool", bufs=2) as hpool,
            tc.tile_pool(name="opool", bufs=2) as opool,
            tc.tile_pool(name="ps_gate", bufs=1, space="PSUM") as ps_gate,
            tc.tile_pool(name="ps_h", bufs=4, space="PSUM") as ps_h,
            tc.tile_pool(name="ps_pre", bufs=2, space="PSUM") as ps_pre,
            tc.tile_pool(name="ps_out", bufs=1, space="PSUM") as ps_out,
        ):
            # ---- load weights/constants once, ordered by first use ----
            wgt = wpool.tile([128, KD * 128], bf16, tag="wg")
            bct = wpool.tile([128, 10], fp32, tag="bct")
            cmt = wpool.tile([128, 128], bf16, tag="cmt")
            wts = [
                wpool.tile([128, KD * 128], bf16, tag=f"w{m}", name=f"w{m}")
                for m in range(MH)
            ]
            vbt = wpool.tile([128, MH * 128], bf16, tag="vbt")
            wc2t = wpool.tile([C1, 128], bf16, tag="wc2t")

            def xdma(t, eng=None):
                # bf16 x chunks on the (otherwise idle) GpSimd DGE queue,
                # consumed directly by the matmuls (no upcast).  Keeping
                # them off the Scalar sequencer matters: DMA programming
                # costs ~590 ns of sequencer time per chunk, which would
                # serialize with the relu/exp ACTIVATE stream.  (Tile 0
                # goes on the Scalar queue instead: the GpSimd DGE is
                # slower to come up at kernel start.)
                xk = []
                for k in range(KD):
                    xb_ = xpool.tile([128, NTILE], bf16, tag=f"xb{k}",
                                     name=f"xb{k}")
                    e = eng[k % len(eng)] if eng else nc.gpsimd
                    e.dma_start(
                        xb_[:],
                        xT[k * 128 : (k + 1) * 128, t * NTILE : (t + 1) * NTILE],
                    )
                    xk.append(xb_)
                return xk

            # PE p-state warm-up: ~3 us of dummy matmuls on zeroed scratch
            # fill the dead window while the first weight/x DMAs land, so
            # the real matmuls start at the full 2.4 GHz clock instead of
            # ramping through the 1.2 GHz mid p-state.
            scr_s = wpool.tile([128, 128], bf16, tag="scr_s")
            scr_m = wpool.tile([128, NTILE], bf16, tag="scr_m")
            nc.vector.memset(scr_s[:], 0.0)
            nc.vector.memset(scr_m[:], 0.0)
            warm = ps_out.tile([128, NTILE], fp32, tag="out")
            for _ in range(26):
                nc.tensor.matmul(warm[:], scr_s[:], scr_m[:], start=True,
                                 stop=True)

            # split the gate-weight preload so the first gate matmul can
            # start as soon as its first k-chunk lands
            for k in range(KD):
                nc.sync.dma_start(
                    wgt[:, k * 128 : (k + 1) * 128],
                    wg[:, k * 128 : (k + 1) * 128],
                )
            xk0 = (
                xdma(0, eng=[nc.scalar, nc.sync]) if repeat == 1 else None
            )
            nc.sync.dma_start(wts[0][:], w1[:, 0 : KD * 128])
            nc.sync.dma_start(bct[:], bcol[:])
            nc.sync.dma_start(cmt[:], cm[:])
            # layer-1 weights m>=1 ride the Scalar queue, which is idle
            # after tile 0's even x chunks (keeps the Sync queue free for
            # the odd x chunks the gate is waiting on)
            for m in range(1, MH):
                nc.scalar.dma_start(
                    wts[m][:], w1[:, m * KD * 128 : (m + 1) * KD * 128]
                )
            nc.sync.dma_start(vbt[:], vb[:])
            nc.sync.dma_start(wc2t[:], wc2[:])

            def gemm_block(wt, pt, xk, stop=True):
                for k in range(KD):
                    nc.tensor.matmul(
                        pt[:], wt[:, k * 128 : (k + 1) * 128], xk[k][:],
                        start=(k == 0), stop=stop and (k == KD - 1),
                    )

            VTAIL = 3  # V matmuls carried across the tile boundary

            def close_prev(prev, t_out):
                """Finish tile t_out: pending V matmuls (closing its pre
                group), then its classifier head front half."""
                pp_p, hs_p = prev
                for vm in range(MH - VTAIL, MH):
                    nc.tensor.matmul(
                        pp_p[:], vbt[:, vm * 128 : (vm + 1) * 128],
                        hs_p[vm][:], start=False, stop=(vm == MH - 1),
                    )
                # reciprocal_approx_fast (custom DVE) misreads PSUM at
                # base partition 64 (HW-verified: values shifted ~2%);
                # relay s through SBUF partitions 0:2 via ACT first.
                sc = spool.tile([NCLS, NTILE], fp32, tag="sc")
                nc.scalar.activation(
                    sc[:], pp_p[C1 : C1 + NCLS, :], AF.Identity
                )
                rv = spool.tile([NCLS, NTILE], fp32, tag="rv")
                nc.vector.reciprocal_approx_fast(rv[:], sc[:])
                rp = spool.tile([C1, NTILE], bf16, tag="rp")
                nc.scalar.activation(rp[:], pp_p[0:C1, :], AF.Relu)
                return rv, rp

            def cls_tail(rv, rp, t_out):
                po = ps_out.tile([128, NTILE], fp32, tag="out")
                nc.tensor.matmul(po[:], wc2t[:], rp[:], start=True, stop=True)
                ot2 = opool.tile([NCLS, NTILE], fp32, tag="o2")
                nc.vector.tensor_tensor(
                    ot2[:], po[0:NCLS, :], rv[:], op=OP.mult
                )
                ot = opool.tile([NCLS, NTILE], fp32, tag="o")
                nc.scalar.activation(
                    ot[:], ot2[:], AF.Identity, bias=bct[0:NCLS, 9:10]
                )
                nc.sync.dma_start(
                    yT[0:NCLS, t_out * NTILE : (t_out + 1) * NTILE], ot[:]
                )

            def batch_loop():
                prev = None  # (pp tile, hs list) of previous btile
                for t in range(nbt):
                    xk = xk0 if (t == 0 and xk0 is not None) else xdma(t)

                    # gate logits (PE)
                    gp = ps_gate.tile([128, NTILE], fp32, tag="gate")
                    gemm_block(wgt, gp, xk)

                    # expg = exp(logit + bg): unnormalized gate weights.
                    # First in the ACT FIFO so the m-loop never waits.
                    expg = spool.tile([128, NTILE], bf16, tag="expg")
                    nc.scalar.activation(expg[:], gp[:], AF.Exp, bias=bct[:, 8:9])

                    # previous tile's pending V matmuls + cls-head front;
                    # by now its hs7 is long since ready, so no PE stall.
                    cls = None
                    if prev is not None:
                        cls = close_prev(prev, t - 1)

                    pp = ps_pre.tile([128, NTILE], fp32, tag="pre")
                    hs = []
                    for m in range(MH):
                        hp = ps_h.tile([128, NTILE], fp32, tag="h")
                        gemm_block(wts[m], hp, xk)
                        if m == 0:
                            # merged matmul opens the pre group:
                            # rows 0:64 = Cm + s*bc1, rows 64:66 = s.
                            nc.tensor.matmul(
                                pp[:], cmt[:], expg[:],
                                start=True, stop=False,
                            )
                            if cls is not None:
                                cls_tail(*cls, t - 1)
                        hr = hpool.tile([128, NTILE], bf16, tag=f"hs{m}",
                                        name=f"hs{m}")
                        nc.scalar.activation(
                            hr[:], hp[:], AF.Relu, bias=bct[:, m : m + 1]
                        )
                        nc.vector.tensor_tensor(
                            hr[:], hr[:], expg[:], op=OP.mult
                        )
                        hs.append(hr)
                        # in-tile V matmuls trail their hs by VTAIL blocks
                        if m >= VTAIL:
                            vm = m - VTAIL
                            nc.tensor.matmul(
                                pp[:], vbt[:, vm * 128 : (vm + 1) * 128],
                                hs[vm][:], start=False, stop=False,
                            )
                    prev = (pp, hs)

                # drain: close the last tile and emit its classifier head
                cls = close_prev(prev, nbt - 1)
                cls_tail(*cls, nbt - 1)

            if repeat > 1:
                with tc.For_i(0, repeat, 1):
                    batch_loop()
            else:
                batch_loop()

    nc.compile()
    return nc


def _get_nc(b_per_core: int, repeat: int = 1):
    key = (b_per_core, repeat)
    if key not in _BUILT:
        _BUILT[key] = _build_nc(b_per_core, repeat)
    return _BUILT[key]


def prep_inputs(x, We1, be1, We2, be2, Wg, bg, Wc1, bc1, Wc2, bc2,
                n_cores=NCORES):
    """Host-side packing -> list of per-core input maps."""
    import ml_dtypes

    f64 = np.float64
    bf16 = ml_dtypes.bfloat16
    b_per_core = x.shape[0] // n_cores

    # layer-1 weights, feature order f = j*E + e
    W1_all = np.transpose(np.asarray(We1, f64), (1, 2, 0)).reshape(D, E * H)
    blocks = []
    for m in range(MH):
        for k in range(KD):
            blocks.append(W1_all[k * 128 : (k + 1) * 128, m * 128 : (m + 1) * 128])
    W1T = np.ascontiguousarray(np.concatenate(blocks, axis=1).astype(bf16))

    Wg_rep = np.asarray(Wg, f64)[:, np.arange(128) % E]
    WGT = np.ascontiguousarray(
        np.concatenate(
            [Wg_rep[k * 128 : (k + 1) * 128, :] for k in range(KD)], axis=1
        ).astype(bf16)
    )

    V = np.einsum(
        "ejk,kc->jec", np.asarray(We2, f64), np.asarray(Wc1, f64)
    ).reshape(E * H, C1)
    # V blocks zero-padded to 128 stationary columns for uniform M=128
    Vb = np.zeros((128, MH * 128), f64)
    for m in range(MH):
        Vb[:, m * 128 : m * 128 + C1] = V[m * 128 : (m + 1) * 128, :]
    Vb = np.ascontiguousarray(Vb.astype(bf16))
    # merged stationary [128, 128], contracted against the mod-8
    # replicated expg (each expert appears 16x -> scale rows by 1/16):
    #   cols 0:64  = (Cm + 1x8 (x) bc1)/16   (C-term + deferred cls bias)
    #   cols 64:66 = 1/16                    (softmax denominator repl.)
    Cm = np.asarray(be2, f64) @ np.asarray(Wc1, f64)  # [E, C1]
    CMB = np.zeros((128, 128), f64)
    rep = np.arange(128) % E
    CMB[:, 0:C1] = (Cm + np.asarray(bc1, f64)[None, :])[rep, :] / 16.0
    CMB[:, C1 : C1 + NCLS] = 1.0 / 16.0
    CMB = np.ascontiguousarray(CMB.astype(bf16))
    WC2 = np.zeros((C1, 128), f64)
    WC2[:, 0:NCLS] = np.asarray(Wc2, f64)
    WC2 = np.ascontiguousarray(WC2.astype(bf16))

    bcol = np.zeros((128, 10), np.float32)
    be1_int = np.asarray(be1, f64).T.reshape(E * H)  # f = j*E + e
    for m in range(MH):
        bcol[:, m] = be1_int[m * 128 : (m + 1) * 128]
    bcol[:, 8] = np.asarray(bg, f64)[np.arange(128) % E]
    bcol[0:NCLS, 9] = np.asarray(bc2, f64)

    xT_full = np.ascontiguousarray(np.asarray(x).T.astype(bf16))  # [D, B]
    in_maps = []
    for c in range(n_cores):
        in_maps.append(
            {
                "xT": np.ascontiguousarray(
                    xT_full[:, c * b_per_core : (c + 1) * b_per_core]
                ),
                "W1T": W1T,
                "WGT": WGT,
                "Vb": Vb,
                "CMB": CMB,
                "WC2": WC2,
                "BCOL": bcol,
            }
        )
    return in_maps, b_per_core


def run(inputs, mm_dt_name=MM_DT, trace=False, repeat=1):
    """Run on 8 NeuronCores; returns (y [B, 2] fp32, exec_time_ns or None)."""
    from concourse.bass_utils import run_bass_kernel_spmd

    in_maps, b_per_core = prep_inputs(**inputs)
    nc = _get_nc(b_per_core, repeat)
    res = run_bass_kernel_spmd(
        nc, in_maps, core_ids=list(range(NCORES)), trace=trace
    )
    y = np.concatenate([r["yT"].T for r in res.results], axis=0)
    return np.ascontiguousarray(y.astype(np.float32)), res.exec_time_ns


def kernel(**inputs):
    y, _ = run(inputs)
    return y


# revision 36
# speedup vs baseline: 1.0222x; 1.0222x over previous
"""Trainium2 Bass kernel for nn_HVGuardModel (dense MoE routing).

Reference math (B=65536, D=1024, E=8, H=128, C1=64, NC=2):
    gw  = softmax(x @ Wg + bg)                      [B, E]
    h   = relu(einsum('bd,edh', x, We1) + be1)      [B, E, H]
    eo  = einsum('beh,eho', h, We2) + be2           [B, E, H]
    mix = einsum('be,beh', gw, eo)                  [B, H]
    out = relu(mix @ Wc1 + bc1) @ Wc2 + bc2         [B, NC]

Strategy: pure data-parallel over 8 cores (8192 rows each), feature-major
[feature, batch] layout, zero device transposes, ALL-BF16 matmuls.

Why all-bf16 (v2 rewrite of the fp32r kernel, measured 351.5us):
  * fp32r matmuls run with fp32_mode=HIGH, which disables the PE's Fast
    Weight Load (EnableFWL requires in_dtype != FP32); the NTFF trace
    showed LDWEIGHTS at ~187 ns/matmul and a steady matmul pace of 233 ns
    vs the 213 ns streaming floor (512 cols @ 2.4 GHz).  bf16 matmuls are
    the same 1 col/cycle but FWL loads weights 2 elems/cycle and the PE's
    64-deep reorder window hides them entirely.
  * The old kernel uploaded x as bf16 and UPCAST to fp32r on DVE (one
    tensor_scalar per chunk).  The trace showed the tile-start gate and
    m=0 matmul stalls (~1 us/tile) all waiting on $S[162] = that DVE
    upcast semaphore, with DVE backed up behind a 3.3 us [64,512]
    RECIPROCAL.  bf16 matmuls consume the DMA'd chunks directly.
  * fp8/DoubleRow is a dead end on this HW: DR is only ~1.44x over bf16
    (LDWEIGHTS +72%, MATMUL +13%), and accuracy needs a hi/lo split that
    multiplies matmul count by >=2.  (Earlier fp32r-session conclusion,
    confirmed by the tensor-engine doc.)

Algebraic folds (host side):
  * V = We2 @ Wc1 per expert ([E*H, 64]) folds expert-2 + gate-mix +
    cls-1 into one PSUM accumulation; eo and mix are never materialized.
  * Layer-1 features interleaved f = j*E + e; a replicated-gate weight
    block (Wg columns tiled mod 8) gives per-partition gate scales with
    no cross-partition broadcast.
  * Softmax division deferred PAST cls-1 via relu(pp/s + bc1) =
    relu(pp + s*bc1)/s  (s > 0): the s*bc1 term rides a single K=128
    "merged" matmul on the replicated expg (stationary rows k = row k%8
    of (Cm + 1x8 (x) bc1), scaled 1/16 since each expert appears 16x)
    which ALSO replicates s itself into pp rows 64:66 (stationary cols
    64:66 = 1/16).  The division shrinks from a [64,512] DVE reciprocal
    (3.3 us!) + [64,512] multiply to a [2,512] reciprocal_approx_fast
    (18-bit exact) + [2,512] multiply on the final classifier output:
    out = (Wc2.T relu(pp'))*(1/s) + bc2.

Uniform matmul shapes: every pre-group matmul is K=128/M=128 -- V blocks
and Wc2 are zero-padded to 128 stationary columns, the merged matmul
contracts over the full replicated expg.  M=64 matmuls measured +190 ns
each (col_grp reconfig, no FWL).  The single accumulation group over pp
rows 0:128 is closed by the last padded V matmul, which is what makes
the in-PSUM s-replication legal.

Hardware quirk (verified by micro-test): reciprocal_approx_fast (custom
DVE op) silently misreads PSUM at base partition 64 -- s is relayed
through SBUF partitions 0:2 via an ACT Identity first.

Schedule (per 512-column batch tile, 82 matmuls):
  * x chunk DMAs ride the otherwise-idle GpSimd DGE queue (tile 0:
    Scalar+SP), weights/outputs SP's; xpool bufs=2.  Keeping DMA
    programming (~590 ns/chunk of sequencer time) off the Scalar
    sequencer stops it serializing with the relu/exp ACTIVATE stream.
  * The classifier head is software-pipelined one tile behind, and the
    last VTAIL=3 V matmuls of each tile are carried across the tile
    boundary (emitted after the next tile's gate matmuls) so the
    h7->relu7->hs7->V7 latency chain (~1.9 us) overlaps the next tile's
    independent gate work instead of stalling PE.
  * ~3 us of dummy matmuls on zeroed scratch warm the PE p-state ramp
    (0.65/1.2 GHz -> 2.4 GHz) while the first weight/x DMAs land.
  * Steady-state pace: 216 ns/matmul = 512 cols @ 2.4 GHz + 2.2 ns
    hwdecode, PE ~92% busy; measured ~314-320 us vs the 373 us fp32r
    baseline.
"""

import numpy as np

B = 65536
D = 1024
E = 8
H = 128
C1 = 64
NCLS = 2
NCORES = 8
BLOC = B // NCORES  # 8192
NTILE = 512
KD = D // 128  # 8 k-chunks over D
MH = (E * H) // 128  # 8 feature blocks

MM_DT = "bfloat16"

_BUILT = {}


def _build_nc(b_per_core: int, repeat: int = 1):
    """repeat > 1 wraps the batch loop in a hardware For_i that re-runs the
    identical work `repeat` times -- used only for wall-clock timing."""
    import concourse.bacc as bacc
    import concourse.tile as tile
    import concourse.mybir as mybir

    nbt = b_per_core // NTILE
    fp32 = mybir.dt.float32
    bf16 = mybir.dt.bfloat16
    AF = mybir.ActivationFunctionType
    OP = mybir.AluOpType

    nc = bacc.Bacc("TRN2", target_bir_lowering=False, debug=False)

    xT = nc.dram_tensor("xT", [D, b_per_core], bf16, kind="ExternalInput")
    w1 = nc.dram_tensor("W1T", [128, MH * KD * 128], bf16, kind="ExternalInput")
    wg = nc.dram_tensor("WGT", [128, KD * 128], bf16, kind="ExternalInput")
    # V blocks padded to 128 stationary columns (cols 64:128 = 0) so every
    # pre-group matmul is a uniform K=128/M=128 shape: M=64 matmuls showed
    # col_grp=h0 array reconfig (+~190 ns each) and no FWL in the trace.
    vb = nc.dram_tensor("Vb", [128, MH * 128], bf16, kind="ExternalInput")
    # K=128 stationary against the mod-8-replicated expg: rows k = row
    # k%8 scaled by 1/16.  cols 0:64 = Cm + 1x8 (x) bc1 (pre term),
    # cols 64:66 = 1 (softmax denominator replicate into pp rows 64:66 --
    # legal as one accumulation group because the padded V matmuls write
    # the full [0:128] partition range), cols 66:128 = 0.
    cm = nc.dram_tensor("CMB", [128, 128], bf16, kind="ExternalInput")
    # Wc2 padded to 128 stationary columns (cols 2:128 = 0) for FWL and a
    # uniform M=128 shape; po rows 2:128 are zeros.
    wc2 = nc.dram_tensor("WC2", [C1, 128], bf16, kind="ExternalInput")
    # per-partition bias columns (fp32): 0..7 = be1 block m, 8 = bg_rep,
    # 9 = bc2 (rows 0:2)
    bcol = nc.dram_tensor("BCOL", [128, 10], fp32, kind="ExternalInput")
    yT = nc.dram_tensor("yT", [NCLS, b_per_core], fp32, kind="ExternalOutput")

    with tile.TileContext(nc) as tc:
        with (
            tc.tile_pool(name="wpool", bufs=1) as wpool,
            tc.tile_pool(name="xpool", bufs=2) as xpool,
            tc.tile_pool(name="spool", bufs=2) as spool,
            tc.tile_pool(name="hpool", bufs=2) as hpool,
            tc.tile_pool(name="opool", bufs=2) as opool,
            tc.tile_pool(name="ps_gate", bufs=1, space="PSUM") as ps_gate,
            tc.tile_pool(name="ps_h", bufs=4, space="PSUM") as ps_h,
            tc.tile_pool(name="ps_pre", bufs=2, space="PSUM") as ps_pre,
            tc.tile_pool(name="ps_out", bufs=1, space="PSUM") as ps_out,
        ):
            # ---- load weights/constants once, ordered by first use ----
            wgt = wpool.tile([128, KD * 128], bf16, tag="wg")
            bct = wpool.tile([128, 10], fp32, tag="bct")
            cmt = wpool.tile([128, 128], bf16, tag="cmt")
            wts = [
                wpool.tile([128, KD * 128], bf16, tag=f"w{m}", name=f"w{m}")
                for m in range(MH)
            ]
            vbt = wpool.tile([128, MH * 128], bf16, tag="vbt")
            wc2t = wpool.tile([C1, 128], bf16, tag="wc2t")

            def xdma(t, eng=None):
                # bf16 x chunks on the (otherwise idle) GpSimd DGE queue,
                # consumed directly by the matmuls (no upcast).  Keeping
                # them off the Scalar sequencer matters: DMA programming
                # costs ~590 ns of sequencer time per chunk, which would
                # serialize with the relu/exp ACTIVATE stream.  (Tile 0
                # goes on the Scalar queue instead: the GpSimd DGE is
                # slower to come up at kernel start.)
                xk = []
                for k in range(KD):
                    xb_ = xpool.tile([128, NTILE], bf16, tag=f"xb{k}",
                                     name=f"xb{k}")
                    e = eng[k % len(eng)] if eng else nc.gpsimd
                    e.dma_start(
                        xb_[:],
                        xT[k * 128 : (k + 1) * 128, t * NTILE : (t + 1) * NTILE],
                    )
                    xk.append(xb_)
                return xk

            # PE p-state warm-up: ~3 us of dummy matmuls on zeroed scratch
            # fill the dead window while the first weight/x DMAs land, so
            # the real matmuls start at the full 2.4 GHz clock instead of
            # ramping through the 1.2 GHz mid p-state.
            scr_s = wpool.tile([128, 128], bf16, tag="scr_s")
            scr_m = wpool.tile([128, NTILE], bf16, tag="scr_m")
            nc.vector.memset(scr_s[:], 0.0)
            nc.vector.memset(scr_m[:], 0.0)
            warm = ps_out.tile([128, NTILE], fp32, tag="out")
            for _ in range(14):
                nc.tensor.matmul(warm[:], scr_s[:], scr_m[:], start=True,
                                 stop=True)

            # split the gate-weight preload so the first gate matmul can
            # start as soon as its first k-chunk lands
            for k in range(KD):
                nc.sync.dma_start(
                    wgt[:, k * 128 : (k + 1) * 128],
                    wg[:, k * 128 : (k + 1) * 128],
                )
            xk0 = (
                xdma(0, eng=[nc.scalar, nc.sync]) if repeat == 1 else None
            )
            nc.sync.dma_start(wts[0][:], w1[:, 0 : KD * 128])
            nc.sync.dma_start(bct[:], bcol[:])
            nc.sync.dma_start(cmt[:], cm[:])
            for m in range(1, MH):
                nc.sync.dma_start(
                    wts[m][:], w1[:, m * KD * 128 : (m + 1) * KD * 128]
                )
            nc.sync.dma_start(vbt[:], vb[:])
            nc.sync.dma_start(wc2t[:], wc2[:])

            def gemm_block(wt, pt, xk, stop=True):
                for k in range(KD):
                    nc.tensor.matmul(
                        pt[:], wt[:, k * 128 : (k + 1) * 128], xk[k][:],
                        start=(k == 0), stop=stop and (k == KD - 1),
                    )

            VTAIL = 3  # V matmuls carried across the tile boundary

            def close_prev(prev, t_out):
                """Finish tile t_out: pending V matmuls (closing its pre
                group), then its classifier head front half."""
                pp_p, hs_p = prev
                for vm in range(MH - VTAIL, MH):
                    nc.tensor.matmul(
                        pp_p[:], vbt[:, vm * 128 : (vm + 1) * 128],
                        hs_p[vm][:], start=False, stop=(vm == MH - 1),
                    )
                # reciprocal_approx_fast (custom DVE) misreads PSUM at
                # base partition 64 (HW-verified: values shifted ~2%);
                # relay s through SBUF partitions 0:2 via ACT first.
                sc = spool.tile([NCLS, NTILE], fp32, tag="sc")
                nc.scalar.activation(
                    sc[:], pp_p[C1 : C1 + NCLS, :], AF.Identity
                )
                rv = spool.tile([NCLS, NTILE], fp32, tag="rv")
                nc.vector.reciprocal_approx_fast(rv[:], sc[:])
                rp = spool.tile([C1, NTILE], bf16, tag="rp")
                nc.scalar.activation(rp[:], pp_p[0:C1, :], AF.Relu)
                return rv, rp

            def cls_tail(rv, rp, t_out):
                po = ps_out.tile([128, NTILE], fp32, tag="out")
                nc.tensor.matmul(po[:], wc2t[:], rp[:], start=True, stop=True)
                ot2 = opool.tile([NCLS, NTILE], fp32, tag="o2")
                nc.vector.tensor_tensor(
                    ot2[:], po[0:NCLS, :], rv[:], op=OP.mult
                )
                ot = opool.tile([NCLS, NTILE], fp32, tag="o")
                nc.scalar.activation(
                    ot[:], ot2[:], AF.Identity, bias=bct[0:NCLS, 9:10]
                )
                nc.sync.dma_start(
                    yT[0:NCLS, t_out * NTILE : (t_out + 1) * NTILE], ot[:]
                )

            def batch_loop():
                prev = None  # (pp tile, hs list) of previous btile
                for t in range(nbt):
                    xk = xk0 if (t == 0 and xk0 is not None) else xdma(t)

                    # gate logits (PE)
                    gp = ps_gate.tile([128, NTILE], fp32, tag="gate")
                    gemm_block(wgt, gp, xk)

                    # expg = exp(logit + bg): unnormalized gate weights.
                    # First in the ACT FIFO so the m-loop never waits.
                    expg = spool.tile([128, NTILE], bf16, tag="expg")
                    nc.scalar.activation(expg[:], gp[:], AF.Exp, bias=bct[:, 8:9])

                    # previous tile's pending V matmuls + cls-head front;
                    # by now its hs7 is long since ready, so no PE stall.
                    cls = None
                    if prev is not None:
                        cls = close_prev(prev, t - 1)

                    pp = ps_pre.tile([128, NTILE], fp32, tag="pre")
                    hs = []
                    for m in range(MH):
                        hp = ps_h.tile([128, NTILE], fp32, tag="h")
                        gemm_block(wts[m], hp, xk)
                        if m == 0:
                            # merged matmul opens the pre group:
                            # rows 0:64 = Cm + s*bc1, rows 64:66 = s.
                            nc.tensor.matmul(
                                pp[:], cmt[:], expg[:],
                                start=True, stop=False,
                            )
                            if cls is not None:
                                cls_tail(*cls, t - 1)
                        hr = hpool.tile([128, NTILE], bf16, tag=f"hs{m}",
                                        name=f"hs{m}")
                        nc.scalar.activation(
                            hr[:], hp[:], AF.Relu, bias=bct[:, m : m + 1]
                        )
                        nc.vector.tensor_tensor(
                            hr[:], hr[:], expg[:], op=OP.mult
                        )
                        hs.append(hr)
                        # in-tile V matmuls trail their hs by VTAIL blocks
                        if m >= VTAIL:
                            vm = m - VTAIL
                            nc.tensor.matmul(
                                pp[:], vbt[:, vm * 128 : (vm + 1) * 128],
                                hs[vm][:], start=False, stop=False,
                            )
                    prev = (pp, hs)

                # drain: close the last tile and emit its classifier head
                cls = close_prev(prev, nbt - 1)
                cls_tail(*cls, nbt - 1)

            if repeat > 1:
                with tc.For_i(0, repeat, 1):
                    batch_loop()
            else:
                batch_loop()

    nc.compile()
    return nc


def _get_nc(b_per_core: int, repeat: int = 1):
    key = (b_per_core, repeat)
    if key not in _BUILT:
        _BUILT[key] = _build_nc(b_per_core, repeat)
    return _BUILT[key]


def prep_inputs(x, We1, be1, We2, be2, Wg, bg, Wc1, bc1, Wc2, bc2,
                n_cores=NCORES):
    """Host-side packing -> list of per-core input maps."""
    import ml_dtypes

    f64 = np.float64
    bf16 = ml_dtypes.bfloat16
    b_per_core = x.shape[0] // n_cores

    # layer-1 weights, feature order f = j*E + e
    W1_all = np.transpose(np.asarray(We1, f64), (1, 2, 0)).reshape(D, E * H)
    blocks = []
    for m in range(MH):
        for k in range(KD):
            blocks.append(W1_all[k * 128 : (k + 1) * 128, m * 128 : (m + 1) * 128])
    W1T = np.ascontiguousarray(np.concatenate(blocks, axis=1).astype(bf16))

    Wg_rep = np.asarray(Wg, f64)[:, np.arange(128) % E]
    WGT = np.ascontiguousarray(
        np.concatenate(
            [Wg_rep[k * 128 : (k + 1) * 128, :] for k in range(KD)], axis=1
        ).astype(bf16)
    )

    V = np.einsum(
        "ejk,kc->jec", np.asarray(We2, f64), np.asarray(Wc1, f64)
    ).reshape(E * H, C1)
    # V blocks zero-padded to 128 stationary columns for uniform M=128
    Vb = np.zeros((128, MH * 128), f64)
    for m in range(MH):
        Vb[:, m * 128 : m * 128 + C1] = V[m * 128 : (m + 1) * 128, :]
    Vb = np.ascontiguousarray(Vb.astype(bf16))
    # merged stationary [128, 128], contracted against the mod-8
    # replicated expg (each expert appears 16x -> scale rows by 1/16):
    #   cols 0:64  = (Cm + 1x8 (x) bc1)/16   (C-term + deferred cls bias)
    #   cols 64:66 = 1/16                    (softmax denominator repl.)
    Cm = np.asarray(be2, f64) @ np.asarray(Wc1, f64)  # [E, C1]
    CMB = np.zeros((128, 128), f64)
    rep = np.arange(128) % E
    CMB[:, 0:C1] = (Cm + np.asarray(bc1, f64)[None, :])[rep, :] / 16.0
    CMB[:, C1 : C1 + NCLS] = 1.0 / 16.0
    CMB = np.ascontiguousarray(CMB.astype(bf16))
    WC2 = np.zeros((C1, 128), f64)
    WC2[:, 0:NCLS] = np.asarray(Wc2, f64)
    WC2 = np.ascontiguousarray(WC2.astype(bf16))

    bcol = np.zeros((128, 10), np.float32)
    be1_int = np.asarray(be1, f64).T.reshape(E * H)  # f = j*E + e
    for m in range(MH):
        bcol[:, m] = be1_int[m * 128 : (m + 1) * 128]
    bcol[:, 8] = np.asarray(bg, f64)[np.arange(128) % E]
    bcol[0:NCLS, 9] = np.asarray(bc2, f64)

    xT_full = np.ascontiguousarray(np.asarray(x).T.astype(bf16))  # [D, B]
    in_maps = []
    for c in range(n_cores):
        in_maps.append(
            {
                "xT": np.ascontiguousarray(
                    xT_full[:, c * b_per_core : (c + 1) * b_per_core]
                ),
                "W1T": W1T,
                "WGT": WGT,
                "Vb": Vb,
                "CMB": CMB,
                "WC2": WC2,
                "BCOL": bcol,
            }
        )
    return in_maps, b_per_core


def run(inputs, mm_dt_name=MM_DT, trace=False, repeat=1):
    """Run on 8 NeuronCores; returns (y [B, 2] fp32, exec_time_ns or None)."""
    from concourse.bass_utils import run_bass_kernel_spmd

    in_maps, b_per_core = prep_inputs(**inputs)
    nc = _get_nc(b_per_core, repeat)
    res = run_bass_kernel_spmd(
        nc, in_maps, core_ids=list(range(NCORES)), trace=trace
    )
    y = np.concatenate([r["yT"].T for r in res.results], axis=0)
    return np.ascontiguousarray(y.astype(np.float32)), res.exec_time_ns


def kernel(**inputs):
    y, _ = run(inputs)
    return y


# revision 40
# speedup vs baseline: 1.0303x; 1.0079x over previous
"""Trainium2 Bass kernel for nn_HVGuardModel (dense MoE routing).

Reference math (B=65536, D=1024, E=8, H=128, C1=64, NC=2):
    gw  = softmax(x @ Wg + bg)                      [B, E]
    h   = relu(einsum('bd,edh', x, We1) + be1)      [B, E, H]
    eo  = einsum('beh,eho', h, We2) + be2           [B, E, H]
    mix = einsum('be,beh', gw, eo)                  [B, H]
    out = relu(mix @ Wc1 + bc1) @ Wc2 + bc2         [B, NC]

Strategy: pure data-parallel over 8 cores (8192 rows each), feature-major
[feature, batch] layout, zero device transposes, ALL-BF16 matmuls.

Why all-bf16 (v2 rewrite of the fp32r kernel, measured 351.5us):
  * fp32r matmuls run with fp32_mode=HIGH, which disables the PE's Fast
    Weight Load (EnableFWL requires in_dtype != FP32); the NTFF trace
    showed LDWEIGHTS at ~187 ns/matmul and a steady matmul pace of 233 ns
    vs the 213 ns streaming floor (512 cols @ 2.4 GHz).  bf16 matmuls are
    the same 1 col/cycle but FWL loads weights 2 elems/cycle and the PE's
    64-deep reorder window hides them entirely.
  * The old kernel uploaded x as bf16 and UPCAST to fp32r on DVE (one
    tensor_scalar per chunk).  The trace showed the tile-start gate and
    m=0 matmul stalls (~1 us/tile) all waiting on $S[162] = that DVE
    upcast semaphore, with DVE backed up behind a 3.3 us [64,512]
    RECIPROCAL.  bf16 matmuls consume the DMA'd chunks directly.
  * fp8/DoubleRow is a dead end on this HW: DR is only ~1.44x over bf16
    (LDWEIGHTS +72%, MATMUL +13%), and accuracy needs a hi/lo split that
    multiplies matmul count by >=2.  (Earlier fp32r-session conclusion,
    confirmed by the tensor-engine doc.)

Algebraic folds (host side):
  * V = We2 @ Wc1 per expert ([E*H, 64]) folds expert-2 + gate-mix +
    cls-1 into one PSUM accumulation; eo and mix are never materialized.
  * Layer-1 features interleaved f = j*E + e; a replicated-gate weight
    block (Wg columns tiled mod 8) gives per-partition gate scales with
    no cross-partition broadcast.
  * Softmax division deferred PAST cls-1 via relu(pp/s + bc1) =
    relu(pp + s*bc1)/s  (s > 0): the s*bc1 term rides a single K=128
    "merged" matmul on the replicated expg (stationary rows k = row k%8
    of (Cm + 1x8 (x) bc1), scaled 1/16 since each expert appears 16x)
    which ALSO replicates s itself into pp rows 64:66 (stationary cols
    64:66 = 1/16).  The division shrinks from a [64,512] DVE reciprocal
    (3.3 us!) + [64,512] multiply to a [2,512] reciprocal_approx_fast
    (18-bit exact) + [2,512] multiply on the final classifier output:
    out = (Wc2.T relu(pp'))*(1/s) + bc2.

Uniform matmul shapes: every pre-group matmul is K=128/M=128 -- V blocks
and Wc2 are zero-padded to 128 stationary columns, the merged matmul
contracts over the full replicated expg.  M=64 matmuls measured +190 ns
each (col_grp reconfig, no FWL).  The single accumulation group over pp
rows 0:128 is closed by the last padded V matmul, which is what makes
the in-PSUM s-replication legal.

Hardware quirk (verified by micro-test): reciprocal_approx_fast (custom
DVE op) silently misreads PSUM at base partition 64 -- s is relayed
through SBUF partitions 0:2 via an ACT Identity first.

Schedule (per 512-column batch tile, 82 matmuls):
  * x chunk DMAs ride the otherwise-idle GpSimd DGE queue (tile 0:
    Scalar+SP), weights/outputs SP's; xpool bufs=2.  Keeping DMA
    programming (~590 ns/chunk of sequencer time) off the Scalar
    sequencer stops it serializing with the relu/exp ACTIVATE stream.
  * The classifier head is software-pipelined one tile behind, and the
    last VTAIL=3 V matmuls of each tile are carried across the tile
    boundary (emitted after the next tile's gate matmuls) so the
    h7->relu7->hs7->V7 latency chain (~1.9 us) overlaps the next tile's
    independent gate work instead of stalling PE.
  * ~3 us of dummy matmuls on zeroed scratch warm the PE p-state ramp
    (0.65/1.2 GHz -> 2.4 GHz) while the first weight/x DMAs land.
  * Steady-state pace: 216 ns/matmul = 512 cols @ 2.4 GHz + 2.2 ns
    hwdecode, PE ~92% busy; measured ~314-320 us vs the 373 us fp32r
    baseline.
"""

import numpy as np

B = 65536
D = 1024
E = 8
H = 128
C1 = 64
NCLS = 2
NCORES = 8
BLOC = B // NCORES  # 8192
NTILE = 512
KD = D // 128  # 8 k-chunks over D
MH = (E * H) // 128  # 8 feature blocks

MM_DT = "bfloat16"

_BUILT = {}


def _build_nc(b_per_core: int, repeat: int = 1):
    """repeat > 1 wraps the batch loop in a hardware For_i that re-runs the
    identical work `repeat` times -- used only for wall-clock timing."""
    import concourse.bacc as bacc
    import concourse.tile as tile
    import concourse.mybir as mybir

    nbt = b_per_core // NTILE
    fp32 = mybir.dt.float32
    bf16 = mybir.dt.bfloat16
    AF = mybir.ActivationFunctionType
    OP = mybir.AluOpType

    nc = bacc.Bacc("TRN2", target_bir_lowering=False, debug=False)

    xT = nc.dram_tensor("xT", [D, b_per_core], bf16, kind="ExternalInput")
    w1 = nc.dram_tensor("W1T", [128, MH * KD * 128], bf16, kind="ExternalInput")
    wg = nc.dram_tensor("WGT", [128, KD * 128], bf16, kind="ExternalInput")
    # V blocks padded to 128 stationary columns (cols 64:128 = 0) so every
    # pre-group matmul is a uniform K=128/M=128 shape: M=64 matmuls showed
    # col_grp=h0 array reconfig (+~190 ns each) and no FWL in the trace.
    vb = nc.dram_tensor("Vb", [128, MH * 128], bf16, kind="ExternalInput")
    # K=128 stationary against the mod-8-replicated expg: rows k = row
    # k%8 scaled by 1/16.  cols 0:64 = Cm + 1x8 (x) bc1 (pre term),
    # cols 64:66 = 1 (softmax denominator replicate into pp rows 64:66 --
    # legal as one accumulation group because the padded V matmuls write
    # the full [0:128] partition range), cols 66:128 = 0.
    cm = nc.dram_tensor("CMB", [128, 128], bf16, kind="ExternalInput")
    # Wc2 padded to [128, 128] (rows 64:128 = 0, cols 2:128 = 0) so the
    # po matmul is a uniform K=128/M=128 shape: K=64 matmuls pay a
    # row-group reconfig penalty.  rp = relu(pp[0:128]) is safe because
    # pp rows 64:66 hold s > 0 (relu-invariant) and rows 66:128 are 0,
    # and the zero weight rows null their contribution.
    wc2 = nc.dram_tensor("WC2", [128, 128], bf16, kind="ExternalInput")
    # per-partition bias columns (fp32): 0..7 = be1 block m, 8 = bg_rep,
    # 9 = bc2 (rows 0:2)
    bcol = nc.dram_tensor("BCOL", [128, 10], fp32, kind="ExternalInput")
    yT = nc.dram_tensor("yT", [NCLS, b_per_core], fp32, kind="ExternalOutput")

    with tile.TileContext(nc) as tc:
        with (
            tc.tile_pool(name="wpool", bufs=1) as wpool,
            tc.tile_pool(name="xpool", bufs=2) as xpool,
            tc.tile_pool(name="spool", bufs=2) as spool,
            tc.tile_pool(name="hpool", bufs=2) as hpool,
            tc.tile_pool(name="opool", bufs=2) as opool,
            tc.tile_pool(name="ps_gate", bufs=1, space="PSUM") as ps_gate,
            tc.tile_pool(name="ps_h", bufs=4, space="PSUM") as ps_h,
            tc.tile_pool(name="ps_pre", bufs=2, space="PSUM") as ps_pre,
            tc.tile_pool(name="ps_out", bufs=1, space="PSUM") as ps_out,
        ):
            # ---- load weights/constants once, ordered by first use ----
            wgt = wpool.tile([128, KD * 128], bf16, tag="wg")
            bct = wpool.tile([128, 10], fp32, tag="bct")
            cmt = wpool.tile([128, 128], bf16, tag="cmt")
            wts = [
                wpool.tile([128, KD * 128], bf16, tag=f"w{m}", name=f"w{m}")
                for m in range(MH)
            ]
            vbt = wpool.tile([128, MH * 128], bf16, tag="vbt")
            wc2t = wpool.tile([128, 128], bf16, tag="wc2t")

            def xdma(t, eng=None):
                # bf16 x chunks on the (otherwise idle) GpSimd DGE queue,
                # consumed directly by the matmuls (no upcast).  Keeping
                # them off the Scalar sequencer matters: DMA programming
                # costs ~590 ns of sequencer time per chunk, which would
                # serialize with the relu/exp ACTIVATE stream.  (Tile 0
                # goes on the Scalar queue instead: the GpSimd DGE is
                # slower to come up at kernel start.)
                xk = []
                for k in range(KD):
                    xb_ = xpool.tile([128, NTILE], bf16, tag=f"xb{k}",
                                     name=f"xb{k}")
                    e = eng[k % len(eng)] if eng else nc.gpsimd
                    e.dma_start(
                        xb_[:],
                        xT[k * 128 : (k + 1) * 128, t * NTILE : (t + 1) * NTILE],
                    )
                    xk.append(xb_)
                return xk

            # PE p-state warm-up: ~3 us of dummy matmuls on zeroed scratch
            # fill the dead window while the first weight/x DMAs land, so
            # the real matmuls start at the full 2.4 GHz clock instead of
            # ramping through the 1.2 GHz mid p-state.
            scr_s = wpool.tile([128, 128], bf16, tag="scr_s")
            scr_m = wpool.tile([128, NTILE], bf16, tag="scr_m")
            nc.vector.memset(scr_s[:], 0.0)
            nc.vector.memset(scr_m[:], 0.0)
            warm = ps_out.tile([128, NTILE], fp32, tag="out")
            for _ in range(14):
                nc.tensor.matmul(warm[:], scr_s[:], scr_m[:], start=True,
                                 stop=True)

            # split the gate-weight preload so the first gate matmul can
            # start as soon as its first k-chunk lands
            for k in range(KD):
                nc.sync.dma_start(
                    wgt[:, k * 128 : (k + 1) * 128],
                    wg[:, k * 128 : (k + 1) * 128],
                )
            xk0 = (
                xdma(0, eng=[nc.scalar, nc.sync]) if repeat == 1 else None
            )
            nc.sync.dma_start(wts[0][:], w1[:, 0 : KD * 128])
            nc.sync.dma_start(bct[:], bcol[:])
            nc.sync.dma_start(cmt[:], cm[:])
            for m in range(1, MH):
                nc.sync.dma_start(
                    wts[m][:], w1[:, m * KD * 128 : (m + 1) * KD * 128]
                )
            nc.sync.dma_start(vbt[:], vb[:])
            nc.sync.dma_start(wc2t[:], wc2[:])

            def gemm_block(wt, pt, xk, stop=True):
                for k in range(KD):
                    nc.tensor.matmul(
                        pt[:], wt[:, k * 128 : (k + 1) * 128], xk[k][:],
                        start=(k == 0), stop=stop and (k == KD - 1),
                    )

            VTAIL = 3  # V matmuls carried across the tile boundary

            def close_prev(prev, t_out):
                """Finish tile t_out: pending V matmuls (closing its pre
                group), then its classifier head front half."""
                pp_p, hs_p = prev
                for vm in range(MH - VTAIL, MH):
                    nc.tensor.matmul(
                        pp_p[:], vbt[:, vm * 128 : (vm + 1) * 128],
                        hs_p[vm][:], start=False, stop=(vm == MH - 1),
                    )
                # reciprocal_approx_fast (custom DVE) misreads PSUM at
                # base partition 64 (HW-verified: values shifted ~2%);
                # relay s through SBUF partitions 0:2 via ACT first.
                sc = spool.tile([NCLS, NTILE], fp32, tag="sc")
                nc.scalar.activation(
                    sc[:], pp_p[C1 : C1 + NCLS, :], AF.Identity
                )
                rv = spool.tile([NCLS, NTILE], fp32, tag="rv")
                nc.vector.reciprocal_approx_fast(rv[:], sc[:])
                rp = spool.tile([128, NTILE], bf16, tag="rp")
                nc.scalar.activation(rp[:], pp_p[:], AF.Relu)
                return rv, rp

            def cls_tail(rv, rp, t_out):
                po = ps_out.tile([128, NTILE], fp32, tag="out")
                nc.tensor.matmul(po[:], wc2t[:], rp[:], start=True, stop=True)
                ot2 = opool.tile([NCLS, NTILE], fp32, tag="o2")
                nc.vector.tensor_tensor(
                    ot2[:], po[0:NCLS, :], rv[:], op=OP.mult
                )
                ot = opool.tile([NCLS, NTILE], fp32, tag="o")
                nc.scalar.activation(
                    ot[:], ot2[:], AF.Identity, bias=bct[0:NCLS, 9:10]
                )
                nc.sync.dma_start(
                    yT[0:NCLS, t_out * NTILE : (t_out + 1) * NTILE], ot[:]
                )

            def batch_loop():
                prev = None  # (pp tile, hs list) of previous btile
                for t in range(nbt):
                    xk = xk0 if (t == 0 and xk0 is not None) else xdma(t)

                    # gate logits (PE)
                    gp = ps_gate.tile([128, NTILE], fp32, tag="gate")
                    gemm_block(wgt, gp, xk)

                    # expg = exp(logit + bg): unnormalized gate weights.
                    # First in the ACT FIFO so the m-loop never waits.
                    expg = spool.tile([128, NTILE], bf16, tag="expg")
                    nc.scalar.activation(expg[:], gp[:], AF.Exp, bias=bct[:, 8:9])

                    # previous tile's pending V matmuls + cls-head front;
                    # by now its hs7 is long since ready, so no PE stall.
                    cls = None
                    if prev is not None:
                        cls = close_prev(prev, t - 1)

                    pp = ps_pre.tile([128, NTILE], fp32, tag="pre")
                    hs = []
                    for m in range(MH):
                        hp = ps_h.tile([128, NTILE], fp32, tag="h")
                        gemm_block(wts[m], hp, xk)
                        if m == 0:
                            # merged matmul opens the pre group:
                            # rows 0:64 = Cm + s*bc1, rows 64:66 = s.
                            nc.tensor.matmul(
                                pp[:], cmt[:], expg[:],
                                start=True, stop=False,
                            )
                            if cls is not None:
                                cls_tail(*cls, t - 1)
                        hr = hpool.tile([128, NTILE], bf16, tag=f"hs{m}",
                                        name=f"hs{m}")
                        nc.scalar.activation(
                            hr[:], hp[:], AF.Relu, bias=bct[:, m : m + 1]
                        )
                        nc.vector.tensor_tensor(
                            hr[:], hr[:], expg[:], op=OP.mult
                        )
                        hs.append(hr)
                        # in-tile V matmuls trail their hs by VTAIL blocks
                        if m >= VTAIL:
                            vm = m - VTAIL
                            nc.tensor.matmul(
                                pp[:], vbt[:, vm * 128 : (vm + 1) * 128],
                                hs[vm][:], start=False, stop=False,
                            )
                    prev = (pp, hs)

                # drain: close the last tile and emit its classifier head
                cls = close_prev(prev, nbt - 1)
                cls_tail(*cls, nbt - 1)

            if repeat > 1:
                with tc.For_i(0, repeat, 1):
                    batch_loop()
            else:
                batch_loop()

    nc.compile()
    return nc


def _get_nc(b_per_core: int, repeat: int = 1):
    key = (b_per_core, repeat)
    if key not in _BUILT:
        _BUILT[key] = _build_nc(b_per_core, repeat)
    return _BUILT[key]


def prep_inputs(x, We1, be1, We2, be2, Wg, bg, Wc1, bc1, Wc2, bc2,
                n_cores=NCORES):
    """Host-side packing -> list of per-core input maps."""
    import ml_dtypes

    f64 = np.float64
    bf16 = ml_dtypes.bfloat16
    b_per_core = x.shape[0] // n_cores

    # layer-1 weights, feature order f = j*E + e
    W1_all = np.transpose(np.asarray(We1, f64), (1, 2, 0)).reshape(D, E * H)
    blocks = []
    for m in range(MH):
        for k in range(KD):
            blocks.append(W1_all[k * 128 : (k + 1) * 128, m * 128 : (m + 1) * 128])
    W1T = np.ascontiguousarray(np.concatenate(blocks, axis=1).astype(bf16))

    Wg_rep = np.asarray(Wg, f64)[:, np.arange(128) % E]
    WGT = np.ascontiguousarray(
        np.concatenate(
            [Wg_rep[k * 128 : (k + 1) * 128, :] for k in range(KD)], axis=1
        ).astype(bf16)
    )

    V = np.einsum(
        "ejk,kc->jec", np.asarray(We2, f64), np.asarray(Wc1, f64)
    ).reshape(E * H, C1)
    # V blocks zero-padded to 128 stationary columns for uniform M=128
    Vb = np.zeros((128, MH * 128), f64)
    for m in range(MH):
        Vb[:, m * 128 : m * 128 + C1] = V[m * 128 : (m + 1) * 128, :]
    Vb = np.ascontiguousarray(Vb.astype(bf16))
    # merged stationary [128, 128], contracted against the mod-8
    # replicated expg (each expert appears 16x -> scale rows by 1/16):
    #   cols 0:64  = (Cm + 1x8 (x) bc1)/16   (C-term + deferred cls bias)
    #   cols 64:66 = 1/16                    (softmax denominator repl.)
    Cm = np.asarray(be2, f64) @ np.asarray(Wc1, f64)  # [E, C1]
    CMB = np.zeros((128, 128), f64)
    rep = np.arange(128) % E
    CMB[:, 0:C1] = (Cm + np.asarray(bc1, f64)[None, :])[rep, :] / 16.0
    CMB[:, C1 : C1 + NCLS] = 1.0 / 16.0
    CMB = np.ascontiguousarray(CMB.astype(bf16))
    WC2 = np.zeros((128, 128), f64)
    WC2[0:C1, 0:NCLS] = np.asarray(Wc2, f64)
    WC2 = np.ascontiguousarray(WC2.astype(bf16))

    bcol = np.zeros((128, 10), np.float32)
    be1_int = np.asarray(be1, f64).T.reshape(E * H)  # f = j*E + e
    for m in range(MH):
        bcol[:, m] = be1_int[m * 128 : (m + 1) * 128]
    bcol[:, 8] = np.asarray(bg, f64)[np.arange(128) % E]
    bcol[0:NCLS, 9] = np.asarray(bc2, f64)

    xT_full = np.ascontiguousarray(np.asarray(x).T.astype(bf16))  # [D, B]
    in_maps = []
    for c in range(n_cores):
        in_maps.append(
            {
                "xT": np.ascontiguousarray(
                    xT_full[:, c * b_per_core : (c + 1) * b_per_core]
                ),
                "W1T": W1T,
                "WGT": WGT,
                "Vb": Vb,
                "CMB": CMB,
                "WC2": WC2,
                "BCOL": bcol,
            }
        )
    return in_maps, b_per_core


def run(inputs, mm_dt_name=MM_DT, trace=False, repeat=1):
    """Run on 8 NeuronCores; returns (y [B, 2] fp32, exec_time_ns or None)."""
    from concourse.bass_utils import run_bass_kernel_spmd

    in_maps, b_per_core = prep_inputs(**inputs)
    nc = _get_nc(b_per_core, repeat)
    res = run_bass_kernel_spmd(
        nc, in_maps, core_ids=list(range(NCORES)), trace=trace
    )
    y = np.concatenate([r["yT"].T for r in res.results], axis=0)
    return np.ascontiguousarray(y.astype(np.float32)), res.exec_time_ns


def kernel(**inputs):
    y, _ = run(inputs)
    return y
